# revision 9
# baseline (speedup 1.0000x reference)
"""Trainium2 Bass kernel for DeformConv2D (b=4, c=64, H=W=128, ks=3).

Sharding: 8 cores = (sample s = core//2) x (row-half = core%2). Each core
computes output rows [64*half, 64*half+64) of its sample.

Per-core dataflow (v2):
  1. Load a 74-row halo slice of x (bf16, CHW) into SBUF, zero-padded cols.
  2. Build XD in DRAM, column-major: slot(w, r) = w*74 + r holds the 256B
     pair (x[:, r, w], x[:, r, w+1]) in bf16 HWC. One 512B gather element
     (elem_step=128, elem_size=256) at slot s covers slots s, s+1 = rows
     (r, r+1) x cols (w, w+1) x 64 ch -- all four bilinear corners.
  3. Offset conv on PE in bf16 (9 taps, K=64 matmuls) -> offsets [18, 8192].
  4. PE-transpose offsets to [128 w, 64 t, 18]; DVE coordinate pipeline
     computes bilinear corner weights W4h (bf16) and ONE int16 gather index
     per (pixel, tap), staged to the wrapped-16 dma_gather layout via DRAM.
  5. dma_gather with prepare_only on 4 SWDGE queues (9 gathers per t-chunk,
     one per tap) -- transfers run on parallel DMA rings, Pool engine only
     does descriptor generation.
  6. DVE combine, 3 big ops per (t-chunk, tap) using a stride-0 broadcast
     AP of W4h over the channel dim: prod = G * W4, then two strided adds
     reduce the 4 corners -> xoff[w, t, n, ci].
  7. PE-transpose xoff -> [(n, ci), pix]; final conv = 5 accumulating
     matmuls (K=576 over (n, ci)) -> out [64 co, 128 pix] per row.
"""
import sys
import types
import numpy as np
import ml_dtypes

sys.path.insert(0, "/opt/trn_rl_repo")

BF16 = ml_dtypes.bfloat16
USE_PREP = True
NCORES = 8
NR = 74          # slab rows (local): row r <-> unpadded row h0-5+r
NW = 132         # padded col slots
NSLOT = NW * NR  # col-major: slot = w*NR + r


def _install_ntff_hook():
    if "antenv.axon_hooks" in sys.modules:
        return
    try:
        import antenv
        from trn_agent_boot.trn_boot import _ntff_profile_via_ctypes
    except Exception:
        return
    mod = types.ModuleType("antenv.axon_hooks")
    _hook = [None]
    mod.set_axon_ntff_profile_hook = lambda h: _hook.__setitem__(0, h)
    mod.get_axon_ntff_profile_hook = lambda: _hook[0]
    sys.modules["antenv.axon_hooks"] = mod
    antenv.axon_hooks = mod
    try:
        mod.set_axon_ntff_profile_hook(
            _ntff_profile_via_ctypes("/opt/axon/libaxon_pjrt.so"))
    except Exception:
        mod.set_axon_ntff_profile_hook(None)


_PROGRAM = None


def _build_program():
    global _PROGRAM
    if _PROGRAM is not None:
        return _PROGRAM
    from contextlib import ExitStack
    import concourse.bass as bass
    import concourse.tile as tile
    from concourse import mybir, bacc

    f32 = mybir.dt.float32
    bf16 = mybir.dt.bfloat16
    i16 = mybir.dt.int16
    i32 = mybir.dt.int32
    A = mybir.AluOpType

    nc = bacc.Bacc(num_swdge_queues=4)
    # ---- I/O ----
    xg_p = nc.declare_dram_parameter("xg", [64, NR * 128], bf16, isOutput=False)
    base2_p = nc.declare_dram_parameter("base2", [128, 64 * 18], f32, isOutput=False)
    xsc_p = nc.declare_dram_parameter("xsc", [128, 4], f32, isOutput=False)
    woff_p = nc.declare_dram_parameter("woff", [64, 9 * 18], bf16, isOutput=False)
    wca_p = nc.declare_dram_parameter("wconv_a", [128, 256], bf16, isOutput=False)
    wcb_p = nc.declare_dram_parameter("wconv_b", [64, 64], bf16, isOutput=False)
    idf_p = nc.declare_dram_parameter("ident_f", [128, 128], f32, isOutput=False)
    idb_p = nc.declare_dram_parameter("ident_b", [128, 128], bf16, isOutput=False)
    out_p = nc.declare_dram_parameter("out", [64, 64 * 128], f32, isOutput=True)

    xd = nc.dram_tensor("xd", [NSLOT, 128], bf16)           # gather source
    gstage = nc.dram_tensor("gstage", [16, 8 * 576], i16)   # idx staging

    with tile.TileContext(nc) as tc, ExitStack() as ctx:
        consts = ctx.enter_context(tc.tile_pool(name="consts", bufs=1))
        big = ctx.enter_context(tc.tile_pool(name="big", bufs=1))
        scratch = ctx.enter_context(tc.tile_pool(name="scratch", bufs=4))
        ps_seq = ctx.enter_context(tc.tile_pool(name="ps_seq", bufs=3, space="PSUM"))
        co_ctx = ExitStack()
        coords = co_ctx.enter_context(tc.tile_pool(name="coords", bufs=1))

        # ---------- load constants ----------
        base2 = consts.tile([128, 64 * 18], f32)
        nc.sync.dma_start(out=base2, in_=base2_p[:, :])
        xsc = consts.tile([128, 4], f32)
        nc.sync.dma_start(out=xsc, in_=xsc_p[:, :])
        woff = consts.tile([64, 9, 18], bf16)
        nc.sync.dma_start(out=woff, in_=woff_p[:, :].rearrange("a (t c) -> a t c", t=9))
        wca = consts.tile([128, 256], bf16)
        nc.sync.dma_start(out=wca, in_=wca_p[:, :])
        wcb = consts.tile([64, 64], bf16)
        nc.sync.dma_start(out=wcb, in_=wcb_p[:, :])
        idf = consts.tile([128, 128], f32)
        nc.sync.dma_start(out=idf, in_=idf_p[:, :])
        idb = consts.tile([128, 128], bf16)
        nc.sync.dma_start(out=idb, in_=idb_p[:, :])

        # ---------- phase A: x load + XD build ----------
        ab_ctx = ExitStack()
        abp = ab_ctx.enter_context(tc.tile_pool(name="abp", bufs=1))
        xsb = abp.tile([64, NR, NW], bf16, name="xsb")  # padded CHW slab
        nc.vector.memset(xsb, 0.0)
        nc.sync.dma_start(
            out=xsb[:, :, 1:129],
            in_=xg_p[:, :].rearrange("c (r w) -> c r w", r=NR))

        # zero-fill whole XD (borders + unwritten cols)
        zb = abp.tile([128, NSLOT // 8], bf16, name='zb')       # 9768/8 = 1221 per part
        nc.vector.memset(zb, 0.0)
        for c8 in range(8):
            nc.sync.dma_start(
                out=bass.AP(tensor=xd, offset=c8 * (NSLOT // 8),
                            ap=[[NSLOT, 128], [1, NSLOT // 8]]),
                in_=zb)

        # transpose x rows -> bf16 HWC, write col pair halves into XD
        for blk in range(10):                        # 8 rows per block; 74 rows
            rows = min(8, NR - blk * 8)
            pst = ps_seq.tile([128, 512], bf16, tag="seq")
            for j in range(rows):
                k = blk * 8 + j
                nc.tensor.transpose(
                    pst[:, j * 64:(j + 1) * 64], xsb[:, k, 1:129],
                    idb[0:64, 0:64])
            xrb = scratch.tile([128, 8, 64], bf16, tag="xrow")
            nc.any.tensor_copy(
                xrb[:, 0:rows, :],
                pst[:, 0:rows * 64].rearrange("p (r c) -> p r c", r=rows))
            # XD[(wp+1)*NR + k, 0:64] = x[., k, wp]   (wp = x col = part idx)
            nc.sync.dma_start(
                out=bass.AP(tensor=xd,
                            offset=(NR + blk * 8) * 128 + 0,
                            ap=[[NR * 128, 128], [128, rows], [1, 64]]),
                in_=xrb[:, 0:rows, :])
            # XD[wp*NR + k, 64:128] = x[., k, wp]
            nc.sync.dma_start(
                out=bass.AP(tensor=xd,
                            offset=(blk * 8) * 128 + 64,
                            ap=[[NR * 128, 128], [128, rows], [1, 64]]),
                in_=xrb[:, 0:rows, :])

        # ---------- phase B: offset conv (bf16) ----------
        off_sb = abp.tile([18, 64, 128], f32, name='off_sb')
        for tb in range(16):                         # 4 output rows per tile
            psc = ps_seq.tile([18, 512], f32, tag="seq")
            for dy in range(3):
                for dx in range(3):
                    tap = dy * 3 + dx
                    nc.tensor.matmul(
                        psc[:, :],
                        woff[:, tap, :],
                        bass.AP(tensor=xsb.tensor,
                                offset=xsb.offset + (tb * 4 + dy + 4) * NW + dx,
                                ap=[xsb.ap[0], [NW, 4], [1, 128]]),
                        start=(tap == 0), stop=(tap == 8))
            nc.any.tensor_copy(
                off_sb[:, tb * 4:tb * 4 + 4, :],
                psc[:, :].rearrange("p (r w) -> p r w", r=4))

        # transpose offsets -> offt [128 w, 64 t, 18]
        offt = coords.tile([128, 64, 18], f32)
        for b in range(4):
            pst = ps_seq.tile([128, 288], f32, tag="seq")
            for j in range(16):
                t = b * 16 + j
                nc.tensor.transpose(
                    pst[:, j * 18:(j + 1) * 18],
                    off_sb[:, t, :], idf[0:18, 0:18])
            nc.any.tensor_copy(
                offt[:, b * 16:(b + 1) * 16, :],
                pst[:, :].rearrange("p (t c) -> p t c", t=16))
        ab_ctx.close()

        # ---------- phase C: coordinates ----------
        def cT(shape, tag):
            return coords.tile(shape, f32, tag=tag, name=tag)

        P = cT([128, 64, 18], "P")
        nc.vector.tensor_tensor(
            P, offt, base2.rearrange("p (t c) -> p t c", t=64), A.add)
        q_i = coords.tile([128, 64, 18], i32, tag="cs", name="qi", bufs=4)
        nc.vector.tensor_copy(q_i, P)
        Qf0 = coords.tile([128, 64, 18], f32, tag="cs", name="qf0", bufs=4)
        nc.vector.tensor_copy(Qf0, q_i)
        GT = coords.tile([128, 64, 18], f32, tag="cs", name="gt", bufs=4)
        nc.vector.tensor_tensor(GT, Qf0, P, A.is_gt)
        Qf = cT([128, 64, 18], "qf")
        nc.vector.tensor_tensor(Qf, Qf0, GT, A.subtract)
        FR = coords.tile([128, 64, 18], f32, tag="cs", name="fr", bufs=4)
        nc.vector.tensor_tensor(FR, P, Qf, A.subtract)
        INR = coords.tile([128, 64, 18], f32, tag="cs", name="inr", bufs=4)
        # x half: per-core bounds via scalar APs; y half: immediates
        nc.vector.tensor_scalar(INR[:, :, 0:9], P[:, :, 0:9],
                                xsc[:, 0:1], None, A.is_ge)
        nc.vector.tensor_scalar(INR[:, :, 9:18], P[:, :, 9:18],
                                9.0, None, A.is_ge)
        INH = coords.tile([128, 64, 18], f32, tag="cs", name="inh", bufs=4)
        nc.vector.tensor_scalar(INH[:, :, 0:9], P[:, :, 0:9],
                                xsc[:, 1:2], None, A.is_le)
        nc.vector.tensor_scalar(INH[:, :, 9:18], P[:, :, 9:18],
                                136.0, None, A.is_le)
        nc.vector.tensor_tensor(INR, INR, INH, A.mult)
        FRV = cT([128, 64, 18], "frv")
        nc.vector.tensor_tensor(FRV, FR, INR, A.mult)
        ALT = cT([128, 64, 18], "alt")
        nc.vector.tensor_scalar(ALT, FRV, -1.0, 1.0, A.mult, A.add)
        QC = cT([128, 64, 18], "qc")
        nc.vector.tensor_scalar(QC[:, :, 0:9], Qf[:, :, 0:9],
                                xsc[:, 2:3], xsc[:, 3:4], A.max, A.min)
        nc.vector.tensor_scalar(QC[:, :, 9:18], Qf[:, :, 9:18],
                                8.0, 137.0, A.max, A.min)
        # gather slot index: slot = (QC_y - 8)*74 + (QC_x - 4)  (f32 exact)
        LINF = cT([128, 64, 9], "linf")
        nc.vector.tensor_scalar(LINF, QC[:, :, 9:18], 74.0, -596.0, A.mult, A.add)
        nc.vector.tensor_tensor(LINF, LINF, QC[:, :, 0:9], A.add)
        gidx_pre = coords.tile([128, 576], i16, tag="gpre", name="gpre")
        nc.vector.tensor_copy(gidx_pre, LINF.rearrange("p a b -> p (a b)"))
        # corner weight products, bf16: [128 w, 64 t, 9 n, 4 rc]
        W4h = consts.tile([128, 64, 9, 4], bf16, tag="w4", name="w4")
        nc.vector.tensor_tensor(W4h[:, :, :, 0], ALT[:, :, 0:9], ALT[:, :, 9:18], A.mult)
        nc.vector.tensor_tensor(W4h[:, :, :, 1], ALT[:, :, 0:9], FRV[:, :, 9:18], A.mult)
        nc.vector.tensor_tensor(W4h[:, :, :, 2], FRV[:, :, 0:9], ALT[:, :, 9:18], A.mult)
        nc.vector.tensor_tensor(W4h[:, :, :, 3], FRV[:, :, 0:9], FRV[:, :, 9:18], A.mult)

        # ---------- idx relayout to wrapped-16 (via DRAM staging) ----------
        # gather idx j = tt*128 + w -> stored at partition w%16,
        # free position tt*8 + (w//16), replicated over the 8 gpsimd cores.
        for ph in range(8):
            sl = gidx_pre[ph * 16:ph * 16 + 16]
            nc.sync.dma_start(
                out=bass.AP(tensor=gstage, offset=ph * 576,
                            ap=[[8 * 576, 16], [1, 576]]),
                in_=bass.AP(tensor=sl.tensor, offset=sl.offset,
                            ap=[sl.ap[0], [1, 576]]))
        sg = consts.tile([128, 8, 576], i16, name="sg")
        nc.gpsimd.dma_start(
            out=sg,
            in_=bass.AP(tensor=gstage, offset=0,
                        ap=[[0, 8], [8 * 576, 16], [1, 8 * 576]]))
        gidx = consts.tile([128, 9, 4, 128], i16)
        # gidx[p, n, tcn, tt*8+ph] = sg[p, ph, (tcn*16+tt)*9 + n]
        for n in range(9):
            nc.vector.tensor_copy(
                bass.AP(tensor=gidx.tensor, offset=gidx.offset + n * 512,
                        ap=[gidx.ap[0], [1, 8], [128, 4], [8, 16]]),
                bass.AP(tensor=sg.tensor, offset=sg.offset + n,
                        ap=[sg.ap[0], [576, 8], [144, 4], [9, 16]]))

        # pre-drain gather deps onto the Pool engine (the DMA-gather ISA
        # struct supports very few semaphore waits)
        j1 = scratch.tile([16, 8], bf16, tag="join", name="j1")
        nc.sync.dma_start(out=j1[0:1, 0:8], in_=xd[0:1, 0:8])
        j2 = scratch.tile([16, 8], i16, tag="join2", name="j2")
        j3 = scratch.tile([16, 8], bf16, tag="join3", name="j3")
        nc.gpsimd.tensor_copy(j2[0:16, 0:4], gidx[0:16, 0, 0, 0:4])
        nc.gpsimd.tensor_copy(j3[0:1, 0:4], j1[0:1, 0:4])

        # ---------- phase D: gather + combine + final conv ----------
        co_ctx.close()
        qsems = [nc.alloc_semaphore(f"gq{q}") for q in range(4)]
        ps_x = ctx.enter_context(tc.tile_pool(name="ps_x", bufs=2, space="PSUM"))
        ps_o = ctx.enter_context(tc.tile_pool(name="ps_o", bufs=2, space="PSUM"))
        gpool = ctx.enter_context(tc.tile_pool(name="gpool", bufs=8))
        xpool = ctx.enter_context(tc.tile_pool(name="xpool", bufs=2))
        spool = ctx.enter_context(tc.tile_pool(name="spool", bufs=2))
        rpool = ctx.enter_context(tc.tile_pool(name="rpool", bufs=3))
        xd_gap = bass.AP(tensor=xd, offset=0, ap=[[128, NSLOT - 1], [1, 256]])
        gi = 0
        qcnt = [0, 0, 0, 0]
        for tcn in range(4):                         # t-chunks of 16 rows
            outb = big.tile([64, 16, 128], f32, tag="outb", bufs=2, name="outb")
            xoff = xpool.tile([128, 16, 9, 64], bf16, tag="xoff", name="xoff")
            gs = []
            gq = []
            for n in range(9):
                g = gpool.tile([128, 16, 2, 2, 64], bf16, tag="g")
                q = gi % 4
                gi += 1
                qcnt[q] += 1
                gq.append((q, qcnt[q]))
                if USE_PREP:
                    nc.gpsimd.dma_gather(
                        out_ap=g.rearrange("p a b c d -> p a (b c d)"),
                        in_ap=xd_gap,
                        idxs_ap=gidx[:, n, tcn, :],
                        num_idxs=2048,
                        num_idxs_reg=2048,
                        elem_size=256,
                        elem_step=128,
                        prepare_only=True,
                        sem=qsems[q],
                        queue_num=q,
                        single_packet=False,
                    )
                    nc.gpsimd.trigger_dma(count=None, queue_num=q)
                else:
                    nc.gpsimd.dma_gather(
                        out_ap=g.rearrange("p a b c d -> p a (b c d)"),
                        in_ap=xd_gap,
                        idxs_ap=gidx[:, n, tcn, :],
                        num_idxs=2048,
                        num_idxs_reg=2048,
                        elem_size=256,
                        elem_step=128,
                        single_packet=False,
                    )
                gs.append(g)
            for n in range(9):
                g4 = gs[n].rearrange("p a b c d -> p a (b c) d")  # [128,16,4,64]
                w4bc = bass.AP(
                    tensor=W4h.tensor,
                    offset=W4h.offset + (tcn * 16) * 36 + n * 4,
                    ap=[W4h.ap[0], [36, 16], [1, 4], [0, 64]])
                mul = nc.vector.tensor_tensor(g4, g4, w4bc, A.mult)
                if USE_PREP:
                    # Tile's auto-gating doesn't cover prepare_only DMA
                    # completion; wait on the descriptor-baked queue sem.
                    q, k = gq[n]
                    mul._wait_ge(qsems[q], 16 * k)
                s2 = spool.tile([128, 16, 2, 64], bf16, tag="s2")
                nc.vector.tensor_tensor(
                    s2, gs[n][:, :, 0, :, :], gs[n][:, :, 1, :, :], A.add)
                nc.vector.tensor_tensor(
                    xoff[:, :, n, :], s2[:, :, 0, :], s2[:, :, 1, :], A.add)
            # transpose xoff per row, final conv
            for tt in range(16):
                pso = ps_o.tile([64, 128], f32, tag="o")
                for jc in range(4):
                    psx = ps_x.tile([128, 128], bf16, tag="x")
                    nc.tensor.transpose(
                        psx,
                        xoff[:, tt, 2 * jc:2 * jc + 2, :].rearrange(
                            "p a b -> p (a b)"),
                        idb)
                    rhs = rpool.tile([128, 128], bf16, tag="r")
                    nc.any.tensor_copy(rhs, psx)
                    nc.tensor.matmul(pso, wca[:, jc * 64:(jc + 1) * 64], rhs,
                                     start=(jc == 0), stop=False)
                psx4 = ps_x.tile([128, 128], bf16, tag="x")
                nc.tensor.transpose(
                    psx4[0:64, :], xoff[:, tt, 8, :], idb)
                rhs4 = rpool.tile([64, 128], bf16, tag="r4")
                nc.any.tensor_copy(rhs4, psx4[0:64, :])
                nc.tensor.matmul(pso, wcb, rhs4, start=False, stop=True)
                nc.any.tensor_copy(outb[:, tt, :], pso)

            nc.sync.dma_start(
                out=out_p[:, tcn * 2048:(tcn + 1) * 2048],
                in_=outb.rearrange("c t w -> c (t w)"))

    nc.finalize()
    _PROGRAM = nc
    return nc


def _host_consts(W_off, b_off, W_conv):
    idxr = np.concatenate([np.arange(0, 18, 2), np.arange(1, 18, 2)])
    W_off_r = W_off[idxr]            # (18, 64, 3, 3)
    b_off_r = b_off[idxr]            # (18,)
    woff = np.ascontiguousarray(
        W_off_r.transpose(2, 3, 1, 0).reshape(9, 64, 18).transpose(1, 0, 2)
    ).reshape(64, 9 * 18).astype(BF16)
    # base2 [128 w, 64 t, 18]
    nidx = np.arange(9)
    pnx = (nidx // 3) - 1
    pny = (nidx % 3) - 1
    tt = np.arange(64)
    ww = np.arange(128)
    base2 = np.zeros((128, 64, 18), np.float32)
    base2[:, :, 0:9] = tt[None, :, None] + 9 + pnx[None, None, :] + \
        b_off_r[None, None, 0:9]
    base2[:, :, 9:18] = ww[:, None, None] + 9 + pny[None, None, :] + \
        b_off_r[None, None, 9:18]
    base2 = base2.reshape(128, 64 * 18)
    # final conv weights
    Wmat = W_conv.reshape(64, 64, 9).transpose(0, 2, 1)   # (co, n, ci)
    wca = np.zeros((128, 256), np.float32)
    for jc in range(4):
        for dn in range(2):
            # K row = dn*64+ci ; col block jc : [K, co]
            wca[dn * 64:(dn + 1) * 64, jc * 64:(jc + 1) * 64] = \
                Wmat[:, 2 * jc + dn, :].T
    wcb = np.ascontiguousarray(Wmat[:, 8, :].T)           # (ci, co)
    return {
        "woff": woff,
        "base2": base2,
        "wconv_a": wca.astype(BF16),
        "wconv_b": wcb.astype(BF16),
        "ident_f": np.eye(128, dtype=np.float32),
        "ident_b": np.eye(128, dtype=np.float32).astype(BF16),
    }


def _per_core_inputs(x, consts, s, half):
    h0 = 64 * half
    xs = x[s]                                    # (64, 128, 128)
    xgs = np.zeros((64, NR, 128), np.float32)
    lo = h0 - 5                                  # unpadded row of xg row 0
    for k in range(NR):
        r = lo + k
        if 0 <= r < 128:
            xgs[:, k, :] = xs[:, r, :]
    xsc = np.zeros((128, 4), np.float32)
    xsc[:, 0] = 9 - h0                           # mask lo
    xsc[:, 1] = 136 - h0                         # mask hi
    xsc[:, 2] = 8 - min(h0, 2)                   # clip lo (tightened)
    xsc[:, 3] = min(min(129, h0 + 69) - h0 + 8, 76)  # clip hi (row+1 in slab)
    return {
        "xg": xgs.reshape(64, NR * 128).astype(BF16),
        "xsc": xsc,
        **consts,
    }


def kernel(x, W_off, b_off, W_conv):
    _install_ntff_hook()
    # the bass kernel must run on the axon trn2 backend; undo any cpu pin
    # (e.g. a harness that set JAX_PLATFORMS=cpu for the reference)
    import os
    if os.environ.get("JAX_PLATFORMS", "") == "cpu":
        try:
            import jax
            jax.config.update("jax_platforms", None)
            os.environ.pop("JAX_PLATFORMS", None)
        except Exception:
            pass
    x = np.asarray(x, np.float32)
    W_off = np.asarray(W_off, np.float32)
    b_off = np.asarray(b_off, np.float32)
    W_conv = np.asarray(W_conv, np.float32)

    from concourse.bass_utils import run_bass_kernel_spmd
    nc = _build_program()
    consts = _host_consts(W_off, b_off, W_conv)
    in_maps = [
        _per_core_inputs(x, consts, core // 2, core % 2) for core in range(NCORES)
    ]
    res = run_bass_kernel_spmd(nc, in_maps, list(range(NCORES)))
    out = np.empty((4, 64, 128, 128), np.float32)
    for core in range(NCORES):
        s, half = core // 2, core % 2
        out[s, :, 64 * half:64 * half + 64, :] = \
            res.results[core]["out"].reshape(64, 64, 128)
    return out


# revision 14
# speedup vs baseline: 1.1224x; 1.1224x over previous
"""Trainium2 Bass kernel for DeformConv2D (b=4, c=64, H=W=128, ks=3).

Sharding: 8 cores = (sample s = core//2) x (row-half = core%2). Each core
computes output rows [64*half, 64*half+64) of its sample.

Per-core dataflow (v2):
  1. Load a 74-row halo slice of x (bf16, CHW) into SBUF, zero-padded cols.
  2. Build XD in DRAM, column-major: slot(w, r) = w*74 + r holds the 256B
     pair (x[:, r, w], x[:, r, w+1]) in bf16 HWC. One 512B gather element
     (elem_step=128, elem_size=256) at slot s covers slots s, s+1 = rows
     (r, r+1) x cols (w, w+1) x 64 ch -- all four bilinear corners.
  3. Offset conv on PE in bf16 (9 taps, K=64 matmuls) -> offsets [18, 8192].
  4. PE-transpose offsets to [128 w, 64 t, 18]; DVE coordinate pipeline
     computes bilinear corner weights W4h (bf16) and ONE int16 gather index
     per (pixel, tap), staged to the wrapped-16 dma_gather layout via DRAM.
  5. dma_gather with prepare_only on 4 SWDGE queues (9 gathers per t-chunk,
     one per tap) -- transfers run on parallel DMA rings, Pool engine only
     does descriptor generation.
  6. DVE combine, 3 big ops per (t-chunk, tap) using a stride-0 broadcast
     AP of W4h over the channel dim: prod = G * W4, then two strided adds
     reduce the 4 corners -> xoff[w, t, n, ci].
  7. PE-transpose xoff -> [(n, ci), pix]; final conv = 5 accumulating
     matmuls (K=576 over (n, ci)) -> out [64 co, 128 pix] per row.
"""
import sys
import types
import numpy as np
import ml_dtypes

sys.path.insert(0, "/opt/trn_rl_repo")

BF16 = ml_dtypes.bfloat16
USE_PREP = False
NCORES = 8
NR = 74          # slab rows (local): row r <-> unpadded row h0-5+r
NW = 132         # padded col slots
NSLOT = NW * NR  # col-major: slot = w*NR + r


def _install_ntff_hook():
    if "antenv.axon_hooks" in sys.modules:
        return
    try:
        import antenv
        from trn_agent_boot.trn_boot import _ntff_profile_via_ctypes
    except Exception:
        return
    mod = types.ModuleType("antenv.axon_hooks")
    _hook = [None]
    mod.set_axon_ntff_profile_hook = lambda h: _hook.__setitem__(0, h)
    mod.get_axon_ntff_profile_hook = lambda: _hook[0]
    sys.modules["antenv.axon_hooks"] = mod
    antenv.axon_hooks = mod
    try:
        mod.set_axon_ntff_profile_hook(
            _ntff_profile_via_ctypes("/opt/axon/libaxon_pjrt.so"))
    except Exception:
        mod.set_axon_ntff_profile_hook(None)


_PROGRAM = None


def _build_program():
    global _PROGRAM
    if _PROGRAM is not None:
        return _PROGRAM
    from contextlib import ExitStack
    import concourse.bass as bass
    import concourse.tile as tile
    from concourse import mybir, bacc

    f32 = mybir.dt.float32
    bf16 = mybir.dt.bfloat16
    i16 = mybir.dt.int16
    i32 = mybir.dt.int32
    A = mybir.AluOpType

    nc = bacc.Bacc(num_swdge_queues=4)
    # ---- I/O ----
    xg_p = nc.declare_dram_parameter("xg", [64, NR * 128], bf16, isOutput=False)
    base2_p = nc.declare_dram_parameter("base2", [128, 64 * 18], f32, isOutput=False)
    xsc_p = nc.declare_dram_parameter("xsc", [128, 4], f32, isOutput=False)
    woff_p = nc.declare_dram_parameter("woff", [64, 9 * 18], bf16, isOutput=False)
    wca_p = nc.declare_dram_parameter("wconv_a", [128, 256], bf16, isOutput=False)
    wcb_p = nc.declare_dram_parameter("wconv_b", [64, 64], bf16, isOutput=False)
    idf_p = nc.declare_dram_parameter("ident_f", [128, 128], f32, isOutput=False)
    idb_p = nc.declare_dram_parameter("ident_b", [128, 128], bf16, isOutput=False)
    out_p = nc.declare_dram_parameter("out", [64, 64 * 128], f32, isOutput=True)

    xd = nc.dram_tensor("xd", [NSLOT, 128], bf16)           # gather source
    gstage = nc.dram_tensor("gstage", [16, 8 * 576], i16)   # idx staging

    with tile.TileContext(nc) as tc, ExitStack() as ctx:
        consts = ctx.enter_context(tc.tile_pool(name="consts", bufs=1))
        big = ctx.enter_context(tc.tile_pool(name="big", bufs=1))
        scratch = ctx.enter_context(tc.tile_pool(name="scratch", bufs=4))
        ps_seq = ctx.enter_context(tc.tile_pool(name="ps_seq", bufs=3, space="PSUM"))
        co_ctx = ExitStack()
        coords = co_ctx.enter_context(tc.tile_pool(name="coords", bufs=1))

        # ---------- load constants ----------
        base2 = consts.tile([128, 64 * 18], f32)
        nc.sync.dma_start(out=base2, in_=base2_p[:, :])
        xsc = consts.tile([128, 4], f32)
        nc.sync.dma_start(out=xsc, in_=xsc_p[:, :])
        woff = consts.tile([64, 9, 18], bf16)
        nc.sync.dma_start(out=woff, in_=woff_p[:, :].rearrange("a (t c) -> a t c", t=9))
        wca = consts.tile([128, 256], bf16)
        nc.sync.dma_start(out=wca, in_=wca_p[:, :])
        wcb = consts.tile([64, 64], bf16)
        nc.sync.dma_start(out=wcb, in_=wcb_p[:, :])
        idf = consts.tile([128, 128], f32)
        nc.sync.dma_start(out=idf, in_=idf_p[:, :])
        idb = consts.tile([128, 128], bf16)
        nc.sync.dma_start(out=idb, in_=idb_p[:, :])

        # ---------- phase A: x load + XD build ----------
        ab_ctx = ExitStack()
        abp = ab_ctx.enter_context(tc.tile_pool(name="abp", bufs=1))
        xsb = abp.tile([64, NR, NW], bf16, name="xsb")  # padded CHW slab
        nc.vector.memset(xsb, 0.0)
        nc.sync.dma_start(
            out=xsb[:, :, 1:129],
            in_=xg_p[:, :].rearrange("c (r w) -> c r w", r=NR))

        # zero-fill whole XD (borders + unwritten cols)
        zb = abp.tile([128, NSLOT // 8], bf16, name='zb')       # 9768/8 = 1221 per part
        nc.vector.memset(zb, 0.0)
        for c8 in range(8):
            nc.sync.dma_start(
                out=bass.AP(tensor=xd, offset=c8 * (NSLOT // 8),
                            ap=[[NSLOT, 128], [1, NSLOT // 8]]),
                in_=zb)

        # ---------- phase B: offset conv (bf16), emitted first: it gates
        # the coords -> idx -> gather chain, while the XD build only gates
        # the gather data reads which start later ----------
        off_sb = abp.tile([18, 64, 128], f32, name='off_sb')
        for tb in range(16):                         # 4 output rows per tile
            psc = ps_seq.tile([18, 512], f32, tag="seq")
            for dy in range(3):
                for dx in range(3):
                    tap = dy * 3 + dx
                    nc.tensor.matmul(
                        psc[:, :],
                        woff[:, tap, :],
                        bass.AP(tensor=xsb.tensor,
                                offset=xsb.offset + (tb * 4 + dy + 4) * NW + dx,
                                ap=[xsb.ap[0], [NW, 4], [1, 128]]),
                        start=(tap == 0), stop=(tap == 8))
            nc.any.tensor_copy(
                off_sb[:, tb * 4:tb * 4 + 4, :],
                psc[:, :].rearrange("p (r w) -> p r w", r=4))

        # transpose offsets -> offt [128 w, 64 t, 18]
        offt = coords.tile([128, 64, 18], f32)
        for b in range(4):
            pst = ps_seq.tile([128, 288], f32, tag="seq")
            for j in range(16):
                t = b * 16 + j
                nc.tensor.transpose(
                    pst[:, j * 18:(j + 1) * 18],
                    off_sb[:, t, :], idf[0:18, 0:18])
            nc.any.tensor_copy(
                offt[:, b * 16:(b + 1) * 16, :],
                pst[:, :].rearrange("p (t c) -> p t c", t=16))

        # ---------- phase A2: XD build (PE + DMA, overlaps coords) ----------
        # transpose x rows -> bf16 HWC, write col pair halves into XD
        for blk in range(10):                        # 8 rows per block; 74 rows
            rows = min(8, NR - blk * 8)
            pst = ps_seq.tile([128, 512], bf16, tag="seq")
            for j in range(rows):
                k = blk * 8 + j
                nc.tensor.transpose(
                    pst[:, j * 64:(j + 1) * 64], xsb[:, k, 1:129],
                    idb[0:64, 0:64])
            xrb = scratch.tile([128, 8, 64], bf16, tag="xrow")
            nc.any.tensor_copy(
                xrb[:, 0:rows, :],
                pst[:, 0:rows * 64].rearrange("p (r c) -> p r c", r=rows))
            # XD[(wp+1)*NR + k, 0:64] = x[., k, wp]   (wp = x col = part idx)
            nc.sync.dma_start(
                out=bass.AP(tensor=xd,
                            offset=(NR + blk * 8) * 128 + 0,
                            ap=[[NR * 128, 128], [128, rows], [1, 64]]),
                in_=xrb[:, 0:rows, :])
            # XD[wp*NR + k, 64:128] = x[., k, wp]
            nc.sync.dma_start(
                out=bass.AP(tensor=xd,
                            offset=(blk * 8) * 128 + 64,
                            ap=[[NR * 128, 128], [128, rows], [1, 64]]),
                in_=xrb[:, 0:rows, :])
        ab_ctx.close()

        # ---------- phase C: coordinates ----------
        def cT(shape, tag):
            return coords.tile(shape, f32, tag=tag, name=tag)

        P = cT([128, 64, 18], "P")
        nc.vector.tensor_tensor(
            P, offt, base2.rearrange("p (t c) -> p t c", t=64), A.add)
        q_i = coords.tile([128, 64, 18], i32, tag="cs", name="qi", bufs=4)
        nc.vector.tensor_copy(q_i, P)
        Qf0 = coords.tile([128, 64, 18], f32, tag="cs", name="qf0", bufs=4)
        nc.vector.tensor_copy(Qf0, q_i)
        GT = coords.tile([128, 64, 18], f32, tag="cs", name="gt", bufs=4)
        nc.vector.tensor_tensor(GT, Qf0, P, A.is_gt)
        Qf = cT([128, 64, 18], "qf")
        nc.vector.tensor_tensor(Qf, Qf0, GT, A.subtract)
        FR = coords.tile([128, 64, 18], f32, tag="cs", name="fr", bufs=4)
        nc.vector.tensor_tensor(FR, P, Qf, A.subtract)
        INR = coords.tile([128, 64, 18], f32, tag="cs", name="inr", bufs=4)
        # x half: per-core bounds via scalar APs; y half: immediates
        nc.vector.tensor_scalar(INR[:, :, 0:9], P[:, :, 0:9],
                                xsc[:, 0:1], None, A.is_ge)
        nc.vector.tensor_scalar(INR[:, :, 9:18], P[:, :, 9:18],
                                9.0, None, A.is_ge)
        INH = coords.tile([128, 64, 18], f32, tag="cs", name="inh", bufs=4)
        nc.vector.tensor_scalar(INH[:, :, 0:9], P[:, :, 0:9],
                                xsc[:, 1:2], None, A.is_le)
        nc.vector.tensor_scalar(INH[:, :, 9:18], P[:, :, 9:18],
                                136.0, None, A.is_le)
        nc.vector.tensor_tensor(INR, INR, INH, A.mult)
        FRV = cT([128, 64, 18], "frv")
        nc.vector.tensor_tensor(FRV, FR, INR, A.mult)
        ALT = cT([128, 64, 18], "alt")
        nc.vector.tensor_scalar(ALT, FRV, -1.0, 1.0, A.mult, A.add)
        QC = cT([128, 64, 18], "qc")
        nc.vector.tensor_scalar(QC[:, :, 0:9], Qf[:, :, 0:9],
                                xsc[:, 2:3], xsc[:, 3:4], A.max, A.min)
        nc.vector.tensor_scalar(QC[:, :, 9:18], Qf[:, :, 9:18],
                                8.0, 137.0, A.max, A.min)
        # gather slot index: slot = (QC_y - 8)*74 + (QC_x - 4)  (f32 exact)
        LINF = cT([128, 64, 9], "linf")
        nc.vector.tensor_scalar(LINF, QC[:, :, 9:18], 74.0, -596.0, A.mult, A.add)
        nc.vector.tensor_tensor(LINF, LINF, QC[:, :, 0:9], A.add)
        gidx_pre = coords.tile([128, 576], i16, tag="gpre", name="gpre")
        nc.vector.tensor_copy(gidx_pre, LINF.rearrange("p a b -> p (a b)"))

        # ---------- idx relayout to wrapped-16 (via DRAM staging) ----------
        # gather idx j = tt*128 + w -> stored at partition w%16,
        # free position tt*8 + (w//16), replicated over the 8 gpsimd cores.
        for ph in range(8):
            sl = gidx_pre[ph * 16:ph * 16 + 16]
            nc.sync.dma_start(
                out=bass.AP(tensor=gstage, offset=ph * 576,
                            ap=[[8 * 576, 16], [1, 576]]),
                in_=bass.AP(tensor=sl.tensor, offset=sl.offset,
                            ap=[sl.ap[0], [1, 576]]))
        sg = consts.tile([128, 8, 576], i16, name="sg")
        nc.gpsimd.dma_start(
            out=sg,
            in_=bass.AP(tensor=gstage, offset=0,
                        ap=[[0, 8], [8 * 576, 16], [1, 8 * 576]]))

        # corner weight products, bf16: [128 w, 64 t, 9 n, 4 rc]
        # (emitted after idx staging so the DVE work overlaps the DMA trip)
        W4h = consts.tile([128, 64, 9, 4], bf16, tag="w4", name="w4")
        nc.vector.tensor_tensor(W4h[:, :, :, 0], ALT[:, :, 0:9], ALT[:, :, 9:18], A.mult)
        nc.vector.tensor_tensor(W4h[:, :, :, 1], ALT[:, :, 0:9], FRV[:, :, 9:18], A.mult)
        nc.vector.tensor_tensor(W4h[:, :, :, 2], FRV[:, :, 0:9], ALT[:, :, 9:18], A.mult)
        nc.vector.tensor_tensor(W4h[:, :, :, 3], FRV[:, :, 0:9], FRV[:, :, 9:18], A.mult)

        gidx = consts.tile([128, 9, 4, 128], i16)
        # gidx[p, n, tcn, tt*8+ph] = sg[p, ph, (tcn*16+tt)*9 + n]
        for n in range(9):
            nc.vector.tensor_copy(
                bass.AP(tensor=gidx.tensor, offset=gidx.offset + n * 512,
                        ap=[gidx.ap[0], [1, 8], [128, 4], [8, 16]]),
                bass.AP(tensor=sg.tensor, offset=sg.offset + n,
                        ap=[sg.ap[0], [576, 8], [144, 4], [9, 16]]))

        # pre-drain gather deps onto the Pool engine (the DMA-gather ISA
        # struct supports very few semaphore waits)
        j1 = scratch.tile([16, 8], bf16, tag="join", name="j1")
        nc.sync.dma_start(out=j1[0:1, 0:8], in_=xd[0:1, 0:8])
        j2 = scratch.tile([16, 8], i16, tag="join2", name="j2")
        j3 = scratch.tile([16, 8], bf16, tag="join3", name="j3")
        nc.gpsimd.tensor_copy(j2[0:16, 0:4], gidx[0:16, 0, 0, 0:4])
        nc.gpsimd.tensor_copy(j3[0:1, 0:4], j1[0:1, 0:4])

        # ---------- phase D: gather + combine + final conv ----------
        co_ctx.close()
        qsems = [nc.alloc_semaphore(f"gq{q}") for q in range(4)]
        ps_x = ctx.enter_context(tc.tile_pool(name="ps_x", bufs=2, space="PSUM"))
        ps_o = ctx.enter_context(tc.tile_pool(name="ps_o", bufs=2, space="PSUM"))
        gpool = ctx.enter_context(tc.tile_pool(name="gpool", bufs=8))
        xpool = ctx.enter_context(tc.tile_pool(name="xpool", bufs=2))
        spool = ctx.enter_context(tc.tile_pool(name="spool", bufs=2))
        rpool = ctx.enter_context(tc.tile_pool(name="rpool", bufs=3))
        xd_gap = bass.AP(tensor=xd, offset=0, ap=[[128, NSLOT - 1], [1, 256]])
        gi = 0
        qcnt = [0, 0, 0, 0]
        for tcn in range(4):                         # t-chunks of 16 rows
            outb = big.tile([64, 16, 128], f32, tag="outb", bufs=2, name="outb")
            xoff = xpool.tile([128, 16, 9, 64], bf16, tag="xoff", name="xoff")
            gs = []
            gq = []
            for n in range(9):
                g = gpool.tile([128, 16, 2, 2, 64], bf16, tag="g")
                q = gi % 4
                gi += 1
                qcnt[q] += 1
                gq.append((q, qcnt[q]))
                if USE_PREP:
                    nc.gpsimd.dma_gather(
                        out_ap=g.rearrange("p a b c d -> p a (b c d)"),
                        in_ap=xd_gap,
                        idxs_ap=gidx[:, n, tcn, :],
                        num_idxs=2048,
                        num_idxs_reg=2048,
                        elem_size=256,
                        elem_step=128,
                        prepare_only=True,
                        sem=qsems[q],
                        queue_num=q,
                        single_packet=False,
                    )
                    nc.gpsimd.trigger_dma(count=None, queue_num=q)
                else:
                    nc.gpsimd.dma_gather(
                        out_ap=g.rearrange("p a b c d -> p a (b c d)"),
                        in_ap=xd_gap,
                        idxs_ap=gidx[:, n, tcn, :],
                        num_idxs=2048,
                        num_idxs_reg=2048,
                        elem_size=256,
                        elem_step=128,
                        single_packet=False,
                    )
                gs.append(g)
            for n in range(9):
                g4 = gs[n].rearrange("p a b c d -> p a (b c) d")  # [128,16,4,64]
                w4bc = bass.AP(
                    tensor=W4h.tensor,
                    offset=W4h.offset + (tcn * 16) * 36 + n * 4,
                    ap=[W4h.ap[0], [36, 16], [1, 4], [0, 64]])
                mul = nc.vector.tensor_tensor(g4, w4bc, g4, A.mult)
                if USE_PREP:
                    # Tile's auto-gating doesn't cover prepare_only DMA
                    # completion; wait on the descriptor-baked queue sem.
                    q, k = gq[n]
                    mul._wait_ge(qsems[q], 16 * k)
                s2 = spool.tile([128, 16, 2, 64], bf16, tag="s2")
                nc.vector.tensor_tensor(
                    s2, gs[n][:, :, 0, :, :], gs[n][:, :, 1, :, :], A.add)
                nc.vector.tensor_tensor(
                    xoff[:, :, n, :], s2[:, :, 0, :], s2[:, :, 1, :], A.add)
            # transpose xoff per row, final conv
            for tt in range(16):
                pso = ps_o.tile([64, 128], f32, tag="o")
                for jc in range(4):
                    psx = ps_x.tile([128, 128], bf16, tag="x")
                    nc.tensor.transpose(
                        psx,
                        xoff[:, tt, 2 * jc:2 * jc + 2, :].rearrange(
                            "p a b -> p (a b)"),
                        idb)
                    rhs = rpool.tile([128, 128], bf16, tag="r")
                    nc.any.tensor_copy(rhs, psx)
                    nc.tensor.matmul(pso, wca[:, jc * 64:(jc + 1) * 64], rhs,
                                     start=(jc == 0), stop=False)
                psx4 = ps_x.tile([128, 128], bf16, tag="x")
                nc.tensor.transpose(
                    psx4[0:64, :], xoff[:, tt, 8, :], idb)
                rhs4 = rpool.tile([64, 128], bf16, tag="r4")
                nc.any.tensor_copy(rhs4, psx4[0:64, :])
                nc.tensor.matmul(pso, wcb, rhs4, start=False, stop=True)
                nc.any.tensor_copy(outb[:, tt, :], pso)

            nc.sync.dma_start(
                out=out_p[:, tcn * 2048:(tcn + 1) * 2048],
                in_=outb.rearrange("c t w -> c (t w)"))

    nc.finalize()
    _PROGRAM = nc
    return nc


def _host_consts(W_off, b_off, W_conv):
    idxr = np.concatenate([np.arange(0, 18, 2), np.arange(1, 18, 2)])
    W_off_r = W_off[idxr]            # (18, 64, 3, 3)
    b_off_r = b_off[idxr]            # (18,)
    woff = np.ascontiguousarray(
        W_off_r.transpose(2, 3, 1, 0).reshape(9, 64, 18).transpose(1, 0, 2)
    ).reshape(64, 9 * 18).astype(BF16)
    # base2 [128 w, 64 t, 18]
    nidx = np.arange(9)
    pnx = (nidx // 3) - 1
    pny = (nidx % 3) - 1
    tt = np.arange(64)
    ww = np.arange(128)
    base2 = np.zeros((128, 64, 18), np.float32)
    base2[:, :, 0:9] = tt[None, :, None] + 9 + pnx[None, None, :] + \
        b_off_r[None, None, 0:9]
    base2[:, :, 9:18] = ww[:, None, None] + 9 + pny[None, None, :] + \
        b_off_r[None, None, 9:18]
    base2 = base2.reshape(128, 64 * 18)
    # final conv weights
    Wmat = W_conv.reshape(64, 64, 9).transpose(0, 2, 1)   # (co, n, ci)
    wca = np.zeros((128, 256), np.float32)
    for jc in range(4):
        for dn in range(2):
            # K row = dn*64+ci ; col block jc : [K, co]
            wca[dn * 64:(dn + 1) * 64, jc * 64:(jc + 1) * 64] = \
                Wmat[:, 2 * jc + dn, :].T
    wcb = np.ascontiguousarray(Wmat[:, 8, :].T)           # (ci, co)
    return {
        "woff": woff,
        "base2": base2,
        "wconv_a": wca.astype(BF16),
        "wconv_b": wcb.astype(BF16),
        "ident_f": np.eye(128, dtype=np.float32),
        "ident_b": np.eye(128, dtype=np.float32).astype(BF16),
    }


def _per_core_inputs(x, consts, s, half):
    h0 = 64 * half
    xs = x[s]                                    # (64, 128, 128)
    xgs = np.zeros((64, NR, 128), np.float32)
    lo = h0 - 5                                  # unpadded row of xg row 0
    for k in range(NR):
        r = lo + k
        if 0 <= r < 128:
            xgs[:, k, :] = xs[:, r, :]
    xsc = np.zeros((128, 4), np.float32)
    xsc[:, 0] = 9 - h0                           # mask lo
    xsc[:, 1] = 136 - h0                         # mask hi
    xsc[:, 2] = 8 - min(h0, 2)                   # clip lo (tightened)
    xsc[:, 3] = min(min(129, h0 + 69) - h0 + 8, 76)  # clip hi (row+1 in slab)
    return {
        "xg": xgs.reshape(64, NR * 128).astype(BF16),
        "xsc": xsc,
        **consts,
    }


def kernel(x, W_off, b_off, W_conv):
    _install_ntff_hook()
    # the bass kernel must run on the axon trn2 backend; undo any cpu pin
    # (e.g. a harness that set JAX_PLATFORMS=cpu for the reference)
    import os
    if os.environ.get("JAX_PLATFORMS", "") == "cpu":
        try:
            import jax
            jax.config.update("jax_platforms", None)
            os.environ.pop("JAX_PLATFORMS", None)
        except Exception:
            pass
    x = np.asarray(x, np.float32)
    W_off = np.asarray(W_off, np.float32)
    b_off = np.asarray(b_off, np.float32)
    W_conv = np.asarray(W_conv, np.float32)

    from concourse.bass_utils import run_bass_kernel_spmd
    nc = _build_program()
    consts = _host_consts(W_off, b_off, W_conv)
    in_maps = [
        _per_core_inputs(x, consts, core // 2, core % 2) for core in range(NCORES)
    ]
    res = run_bass_kernel_spmd(nc, in_maps, list(range(NCORES)))
    out = np.empty((4, 64, 128, 128), np.float32)
    for core in range(NCORES):
        s, half = core // 2, core % 2
        out[s, :, 64 * half:64 * half + 64, :] = \
            res.results[core]["out"].reshape(64, 64, 128)
    return out
